# revision 4
# baseline (speedup 1.0000x reference)
"""ChebConvNet (K=1) Trainium2 kernel: 3x silu(x@W+b) -> logits -> log_softmax.

Sharding: data-parallel over nodes across 8 NeuronCores. x is padded from
200000 to 200704 rows (8 * 25088); each core processes its shard in a
transposed [feat, node] layout so the 128-wide feature dim sits on SBUF
partitions. The scalar (ACT) engine is the bottleneck (~1 col/cycle @
1.2 GHz for every silu/exp element), so the schedule is built to keep it
saturated:

- whole-shard xT staged into SBUF via graduated chunk DMAs spread over
  the sync/tensor/gpsimd HWDGE queues, issued at t=0 so phase A0 is
  never DMA-paced;
- a dummy 1-col silu at program start pulls the silu ACT-table load off
  the critical path (overlaps the first x chunk DMA);
- three silu layers as streaming phases over the shard (1536-col PSUM
  macro tiles everywhere = 3 PSUM banks x2 bufs + 1 bank x2 for the
  z-logit matmuls);
- the z (W3) matmuls run one macro-tile behind the silu stream so the
  PE never makes ACT wait;
- log_softmax interleaves exp and ln per chunk with ZERO table thrash:
  get_activation_tables is patched so walrus must serve both exp and ln
  from the joint natural_log_exp_and_others set (one table switch for
  the whole tail);
- subtract (z - logsumexp) alternates gpsimd/vector, outputs stream out
  over two DMA queues, and the last chunk is small to shrink the tail.

The device writes output partition-major; the host unscrambles.
edge_index is unused (ChebConv with K=1 ignores the graph).
"""

import numpy as np

import concourse.bacc as bacc
import concourse.mybir as mybir
import concourse.tile as tile
from concourse.tile import add_dep_helper
from concourse.bass_utils import run_bass_kernel_spmd

P = 128          # feature dim == SBUF partitions
C = 40           # classes
N_FULL = 200000
N_CORES = 8
NS = 25088       # nodes per core (padded: 8 * 25088 = 200704)
MT = 1536        # macro tile (nodes), 3 PSUM banks
MACROS = [MT] * 16 + [512]            # 16*1536 + 512 = 25088
NG = NS // P                          # 196 node groups per core
# xT staging chunks (cols): small first so the first macro tile lands fast
XCHUNKS = [1024, 1024, 2048, 2048, 3072, 3072, 4096, 4096, 4608]
assert sum(XCHUNKS) == NS
# log_softmax chunks (node groups); small last chunk shrinks the tail
BCHUNKS = [28, 28, 28, 28, 28, 28, 20, 8]
assert sum(BCHUNKS) == NG

F32 = mybir.dt.float32
BF16 = mybir.dt.bfloat16
AF = mybir.ActivationFunctionType

_CACHE = {}


def _patch_act_tables():
    """Force exp and ln to resolve to the joint natural_log_exp set so the
    log_softmax tail needs one ACT table switch instead of four."""
    if getattr(bacc, "_act_tables_patched", False):
        return
    orig = bacc.get_activation_tables

    def patched(arch):
        tabs = {k: set(v) for k, v in orig(arch).items()}
        for name, fns in tabs.items():
            if name != "natural_log_exp_and_others":
                fns.discard(AF.Exp)
                fns.discard(AF.Ln)
        return tabs

    bacc.get_activation_tables = patched
    bacc._act_tables_patched = True


def _build():
    if "nc" in _CACHE:
        return _CACHE["nc"]
    _patch_act_tables()
    nc = bacc.Bacc(None, target_bir_lowering=False)
    xT = nc.declare_dram_parameter("xT", [P, NS], BF16, isOutput=False)
    # consts per partition: 3 W (256B bf16) + W3 (80B) + 3 bias f32 + b3rep
    CB = 3 * 2 * P + 2 * C + 3 * 4 + 4 * 12 * C  # 2780 bytes/partition
    cd = nc.declare_dram_parameter("consts", [P, CB], mybir.dt.uint8, isOutput=False)
    # partition-major scratch layout; host unscrambles to [NS, C]
    out = nc.declare_dram_parameter("out", [P, NG * C], F32, isOutput=True)

    with tile.TileContext(nc) as tc:
        with (
            tc.tile_pool(name="const", bufs=1) as cpool,
            tc.tile_pool(name="big", bufs=1) as bigp,
            tc.tile_pool(name="h2s", bufs=2) as h2sp,
            tc.tile_pool(name="ob", bufs=3) as obp,
            tc.tile_pool(name="ph", bufs=2, space="PSUM") as ph,
            tc.tile_pool(name="pz", bufs=2, space="PSUM") as pz,
        ):
            # ---- t=0: silu table preload via dummy activation ----
            dz = cpool.tile([P, 8], F32, tag="dz")
            nc.vector.memset(dz[:, 0:4], 0.0)
            nc.scalar.activation(dz[:, 4:8], dz[:, 0:4], AF.Silu)

            # ---- consts ----
            craw = cpool.tile([P, CB], mybir.dt.uint8, tag="craw")
            nc.sync.dma_start(craw[:], cd[:])
            off = 0
            Wt = []
            for i in range(3):
                Wt.append(craw[:, off : off + 2 * P].bitcast(BF16))
                off += 2 * P
            W3t = craw[:, off : off + 2 * C].bitcast(BF16)
            off += 2 * C
            bt = []
            for i in range(3):
                bt.append(craw[:, off : off + 4].bitcast(F32))
                off += 4
            b3rep = craw[:, off : off + 4 * 12 * C].bitcast(F32)

            # ---- whole-shard staging; h1 reuses xT's slot, exp reuses h0's
            xall = bigp.tile([P, NS], BF16, tag="bigX", name="xall")
            h0 = bigp.tile([P, NS], BF16, tag="bigA", name="h0all")
            zall = bigp.tile([P, NG * C], F32, tag="zall")
            sall = bigp.tile([P, NG], F32, tag="sall")
            lsall = bigp.tile([P, NG], F32, tag="lsall")

            # ---- xT chunk DMAs across 3 queues, issued up front ----
            qeng = [nc.sync, nc.gpsimd]
            c0 = 0
            for k, cw in enumerate(XCHUNKS):
                qeng[k % 2].dma_start(xall[:, c0 : c0 + cw], xT[:, c0 : c0 + cw])
                c0 += cw

            # ---- Phase A0: h0 = silu(x @ W0 + b0) ----
            n0 = 0
            for mt in MACROS:
                hp = ph.tile([P, MT], F32, tag="hpsum", name="hp0")
                for j in range(0, mt, 512):
                    nc.tensor.matmul(
                        hp[:, j : j + 512], Wt[0], xall[:, n0 + j : n0 + j + 512],
                        start=True, stop=True,
                    )
                nc.scalar.activation(
                    h0[:, n0 : n0 + mt], hp[:, :mt], AF.Silu,
                    bias=bt[0], scale=1.0,
                )
                n0 += mt

            # ---- Phase A1: h1 = silu(h0 @ W1 + b1) (h1 aliases xall) ----
            h1 = bigp.tile([P, NS], BF16, tag="bigX", name="h1all")
            n0 = 0
            for mt in MACROS:
                hp = ph.tile([P, MT], F32, tag="hpsum", name="hp1")
                for j in range(0, mt, 512):
                    nc.tensor.matmul(
                        hp[:, j : j + 512], Wt[1], h0[:, n0 + j : n0 + j + 512],
                        start=True, stop=True,
                    )
                nc.scalar.activation(
                    h1[:, n0 : n0 + mt], hp[:, :mt], AF.Silu,
                    bias=bt[1], scale=1.0,
                )
                n0 += mt

            # ---- Phase A2: h2 = silu(h1 @ W2 + b2); z = h2 @ W3 + b3 ----
            # z-matmuls run one macro tile behind silu so PE never stalls ACT
            pend = None  # (h2_tile, n0, mt) awaiting z-matmuls
            last_silu = None

            def emit_z(h2t, zn0, zmt):
                g0, gn = zn0 // P, zmt // P
                zp = pz.tile([P, 12 * C], F32, tag="zpsum")
                for g in range(gn):
                    nc.tensor.matmul(
                        zp[:, g * C : (g + 1) * C],
                        h2t[:, g * P : (g + 1) * P],
                        W3t,
                        start=True, stop=True,
                    )
                nc.vector.tensor_add(
                    zall[:, g0 * C : (g0 + gn) * C],
                    zp[:, : gn * C],
                    b3rep[:, : gn * C],
                )

            n0 = 0
            for mt in MACROS:
                hp = ph.tile([P, MT], F32, tag="hpsum", name="hp2")
                for j in range(0, mt, 512):
                    nc.tensor.matmul(
                        hp[:, j : j + 512], Wt[2], h1[:, n0 + j : n0 + j + 512],
                        start=True, stop=True,
                    )
                h2 = h2sp.tile([P, MT], BF16, tag="h2")
                last_silu = nc.scalar.activation(
                    h2[:, :mt], hp[:, :mt], AF.Silu, bias=bt[2], scale=1.0
                )
                if pend is not None:
                    emit_z(*pend)
                pend = (h2, n0, mt)
                n0 += mt
            emit_z(*pend)

            # ---- Phase B: log_softmax, exp/ln interleaved (joint table set)
            eall = bigp.tile([P, NG * C], F32, tag="bigA", name="eall")

            exps = []
            g0 = 0
            for gn in BCHUNKS:
                e = nc.scalar.activation(
                    eall[:, g0 * C : (g0 + gn) * C],
                    zall[:, g0 * C : (g0 + gn) * C],
                    AF.Exp,
                )
                exps.append((e, g0, gn))
                g0 += gn
            add_dep_helper(exps[0][0].ins, last_silu.ins, sync=True,
                           reason="exp after all silus (ACT table switch)")

            sub_eng = [nc.gpsimd, nc.vector]
            out_q = [nc.sync, nc.gpsimd]
            for k, (e, g0, gn) in enumerate(exps):
                nc.vector.reduce_sum(
                    sall[:, g0 : g0 + gn],
                    eall[:, g0 * C : (g0 + gn) * C].rearrange(
                        "p (g c) -> p g c", g=gn
                    ),
                    axis=mybir.AxisListType.X,
                )
                nc.scalar.activation(
                    lsall[:, g0 : g0 + gn], sall[:, g0 : g0 + gn], AF.Ln
                )
                o = obp.tile([P, 28 * C], F32, tag="o")
                sub_eng[k % 2].tensor_tensor(
                    o[:, : gn * C].rearrange("p (g c) -> p g c", g=gn),
                    zall[:, g0 * C : (g0 + gn) * C].rearrange(
                        "p (g c) -> p g c", g=gn
                    ),
                    lsall[:, g0 : g0 + gn].broadcast_to([P, gn, C]),
                    op=mybir.AluOpType.subtract,
                )
                out_q[k % 2].dma_start(
                    out[:, g0 * C : (g0 + gn) * C], o[:, : gn * C]
                )
    nc.compile()
    _CACHE["nc"] = nc
    return nc


def _in_maps(x, W0, b0, W1, b1, W2, b2, W3, b3):
    import ml_dtypes

    x = np.asarray(x, dtype=np.float32)
    xpad = np.zeros((N_CORES * NS, P), dtype=ml_dtypes.bfloat16)
    xpad[:N_FULL] = x
    parts = [
        np.asarray(W0, np.float32).astype(ml_dtypes.bfloat16).view(np.uint8),
        np.asarray(W1, np.float32).astype(ml_dtypes.bfloat16).view(np.uint8),
        np.asarray(W2, np.float32).astype(ml_dtypes.bfloat16).view(np.uint8),
        np.asarray(W3, np.float32).astype(ml_dtypes.bfloat16).view(np.uint8),
        np.asarray(b0, np.float32).reshape(P, 1).view(np.uint8),
        np.asarray(b1, np.float32).reshape(P, 1).view(np.uint8),
        np.asarray(b2, np.float32).reshape(P, 1).view(np.uint8),
        np.ascontiguousarray(
            np.broadcast_to(np.tile(np.asarray(b3, np.float32), 12), (P, 12 * C))
        ).view(np.uint8),
    ]
    common = {"consts": np.ascontiguousarray(np.concatenate(parts, axis=1))}
    maps = []
    for c in range(N_CORES):
        shard = xpad[c * NS : (c + 1) * NS]
        maps.append({**common, "xT": np.ascontiguousarray(shard.T)})
    return maps


def _unscramble(res):
    # device out: [128, 196*40] with node = g*128 + p  ->  [25088, 40]
    outs = []
    for c in range(N_CORES):
        o = res.results[c]["out"].reshape(P, NG, C)
        outs.append(np.ascontiguousarray(o.transpose(1, 0, 2)).reshape(NS, C))
    return np.concatenate(outs, axis=0)[:N_FULL]


def kernel(**inputs):
    nc = _build()
    maps = _in_maps(
        inputs["x"],
        inputs["W0"], inputs["b0"],
        inputs["W1"], inputs["b1"],
        inputs["W2"], inputs["b2"],
        inputs["W3"], inputs["b3"],
    )
    res = run_bass_kernel_spmd(nc, maps, list(range(N_CORES)))
    return _unscramble(res)


# revision 5
# speedup vs baseline: 1.0228x; 1.0228x over previous
"""ChebConvNet (K=1) Trainium2 kernel: 3x silu(x@W+b) -> logits -> log_softmax.

Sharding: data-parallel over nodes across 8 NeuronCores. x is padded from
200000 to 200704 rows (8 * 25088); each core processes its shard in a
transposed [feat, node] layout so the 128-wide feature dim sits on SBUF
partitions. The scalar (ACT) engine is the bottleneck (~1 col/cycle @
1.2 GHz for every silu/exp element), so the schedule is built to keep it
saturated:

- whole-shard xT staged into SBUF: early columns stream as small chunks
  on the sync HWDGE queue, late columns as big chunks on the scalar
  HWDGE queue, all issued at t=0 so phase A0 is never DMA-paced;
- a dummy 1-col silu at program start (fed by a scalar-engine memzero,
  so no cross-engine dependency) pulls the silu ACT-table load off the
  critical path;
- three silu layers as streaming phases over the shard (1536-col PSUM
  macro tiles = 3 PSUM banks x2 bufs + 1 bank x2 for the z matmuls);
- the z (W3) matmuls run one macro-tile behind the silu stream so the
  PE never makes ACT wait;
- log_softmax interleaves exp and ln per chunk with ZERO table thrash:
  get_activation_tables is patched so walrus must serve both exp and ln
  from the joint natural_log_exp_and_others set (one table switch);
- z and the output are bf16 (rel err ~4e-3, budget 2e-2), halving the
  output DMA; subtract (z - logsumexp) runs gpsimd-heavy with the tail
  chunks on vector; outputs stream out over the sync + scalar queues.

The device writes output partition-major; the host unscrambles and
upcasts to f32. edge_index is unused (ChebConv with K=1 ignores it).
"""

import numpy as np

import concourse.bacc as bacc
import concourse.mybir as mybir
import concourse.tile as tile
from concourse.tile import add_dep_helper
from concourse.bass_utils import run_bass_kernel_spmd

P = 128          # feature dim == SBUF partitions
C = 40           # classes
N_FULL = 200000
N_CORES = 8
NS = 25088       # nodes per core (padded: 8 * 25088 = 200704)
MT = 1536        # macro tile (nodes), 3 PSUM banks
MACROS = [MT] * 16 + [512]            # 16*1536 + 512 = 25088
NG = NS // P                          # 196 node groups per core
# xT staging chunks: (cols, queue) — queue 0 = sync, 1 = scalar.
# Early cols small+sync for a fast A0 start; late cols big on scalar.
XCHUNKS = [
    (512, 0), (512, 0), (1024, 0), (1024, 0), (2048, 0), (2048, 0), (3072, 0),
    (4096, 1), (4096, 1), (6656, 1),
]
assert sum(c for c, _ in XCHUNKS) == NS
# log_softmax chunks (node groups); small last chunks shrink the tail.
# (groups, subtract engine 0=gpsimd 1=vector, out queue 0=sync 1=scalar)
BCHUNKS = [
    (28, 0, 0), (28, 0, 1), (28, 0, 0), (28, 0, 1),
    (28, 1, 0), (28, 0, 1), (20, 1, 0), (8, 0, 1),
]
assert sum(g for g, _, _ in BCHUNKS) == NG

F32 = mybir.dt.float32
BF16 = mybir.dt.bfloat16
AF = mybir.ActivationFunctionType

_CACHE = {}


def _patch_act_tables():
    """Force exp and ln to resolve to the joint natural_log_exp set so the
    log_softmax tail needs one ACT table switch instead of four."""
    if getattr(bacc, "_act_tables_patched", False):
        return
    orig = bacc.get_activation_tables

    def patched(arch):
        tabs = {k: set(v) for k, v in orig(arch).items()}
        for name, fns in tabs.items():
            if name != "natural_log_exp_and_others":
                fns.discard(AF.Exp)
                fns.discard(AF.Ln)
        return tabs

    bacc.get_activation_tables = patched
    bacc._act_tables_patched = True


def _build():
    if "nc" in _CACHE:
        return _CACHE["nc"]
    _patch_act_tables()
    nc = bacc.Bacc(None, target_bir_lowering=False)
    xT = nc.declare_dram_parameter("xT", [P, NS], BF16, isOutput=False)
    # consts per partition: 3 W (256B bf16) + W3 (80B) + 3 bias f32 + b3rep
    CB = 3 * 2 * P + 2 * C + 3 * 4 + 4 * 12 * C  # 2780 bytes/partition
    cd = nc.declare_dram_parameter("consts", [P, CB], mybir.dt.uint8, isOutput=False)
    # partition-major scratch layout; host unscrambles to [NS, C]
    out = nc.declare_dram_parameter("out", [P, NG * C], BF16, isOutput=True)

    with tile.TileContext(nc) as tc:
        with (
            tc.tile_pool(name="const", bufs=1) as cpool,
            tc.tile_pool(name="big", bufs=1) as bigp,
            tc.tile_pool(name="h2s", bufs=2) as h2sp,
            tc.tile_pool(name="ob", bufs=3) as obp,
            tc.tile_pool(name="ph", bufs=2, space="PSUM") as ph,
            tc.tile_pool(name="pz", bufs=2, space="PSUM") as pz,
        ):
            # ---- t=0: silu table preload via dummy activation (scalar-only)
            dz = cpool.tile([P, 8], F32, tag="dz")
            nc.scalar.memzero(dz[:, 0:4])
            nc.scalar.activation(dz[:, 4:8], dz[:, 0:4], AF.Silu)

            # ---- consts ----
            craw = cpool.tile([P, CB], mybir.dt.uint8, tag="craw")
            nc.sync.dma_start(craw[:], cd[:])
            off = 0
            Wt = []
            for i in range(3):
                Wt.append(craw[:, off : off + 2 * P].bitcast(BF16))
                off += 2 * P
            W3t = craw[:, off : off + 2 * C].bitcast(BF16)
            off += 2 * C
            bt = []
            for i in range(3):
                bt.append(craw[:, off : off + 4].bitcast(F32))
                off += 4
            b3rep = craw[:, off : off + 4 * 12 * C].bitcast(F32)

            # ---- whole-shard staging; h1 reuses xT's slot, exp reuses h0's
            xall = bigp.tile([P, NS], BF16, tag="bigX", name="xall")
            h0 = bigp.tile([P, NS], BF16, tag="bigA", name="h0all")
            zall = bigp.tile([P, NG * C], BF16, tag="zall")
            sall = bigp.tile([P, NG], F32, tag="sall")
            lsall = bigp.tile([P, NG], F32, tag="lsall")

            # ---- xT chunk DMAs across sync + scalar HWDGE queues ----
            qeng = [nc.sync, nc.scalar]
            c0 = 0
            for cw, q in XCHUNKS:
                qeng[q].dma_start(xall[:, c0 : c0 + cw], xT[:, c0 : c0 + cw])
                c0 += cw

            # ---- Phase A0: h0 = silu(x @ W0 + b0) ----
            n0 = 0
            for mt in MACROS:
                hp = ph.tile([P, MT], F32, tag="hpsum", name="hp0")
                for j in range(0, mt, 512):
                    nc.tensor.matmul(
                        hp[:, j : j + 512], Wt[0], xall[:, n0 + j : n0 + j + 512],
                        start=True, stop=True,
                    )
                nc.scalar.activation(
                    h0[:, n0 : n0 + mt], hp[:, :mt], AF.Silu,
                    bias=bt[0], scale=1.0,
                )
                n0 += mt

            # ---- Phase A1: h1 = silu(h0 @ W1 + b1) (h1 aliases xall) ----
            h1 = bigp.tile([P, NS], BF16, tag="bigX", name="h1all")
            n0 = 0
            for mt in MACROS:
                hp = ph.tile([P, MT], F32, tag="hpsum", name="hp1")
                for j in range(0, mt, 512):
                    nc.tensor.matmul(
                        hp[:, j : j + 512], Wt[1], h0[:, n0 + j : n0 + j + 512],
                        start=True, stop=True,
                    )
                nc.scalar.activation(
                    h1[:, n0 : n0 + mt], hp[:, :mt], AF.Silu,
                    bias=bt[1], scale=1.0,
                )
                n0 += mt

            # ---- Phase A2: h2 = silu(h1 @ W2 + b2); z = h2 @ W3 + b3 ----
            # z-matmuls run one macro tile behind silu so PE never stalls ACT
            pend = None  # (h2_tile, n0, mt) awaiting z-matmuls
            last_silu = None

            def emit_z(h2t, zn0, zmt):
                g0, gn = zn0 // P, zmt // P
                zp = pz.tile([P, 12 * C], F32, tag="zpsum")
                for g in range(gn):
                    nc.tensor.matmul(
                        zp[:, g * C : (g + 1) * C],
                        h2t[:, g * P : (g + 1) * P],
                        W3t,
                        start=True, stop=True,
                    )
                nc.vector.tensor_add(
                    zall[:, g0 * C : (g0 + gn) * C],
                    zp[:, : gn * C],
                    b3rep[:, : gn * C],
                )

            n0 = 0
            for mt in MACROS:
                hp = ph.tile([P, MT], F32, tag="hpsum", name="hp2")
                for j in range(0, mt, 512):
                    nc.tensor.matmul(
                        hp[:, j : j + 512], Wt[2], h1[:, n0 + j : n0 + j + 512],
                        start=True, stop=True,
                    )
                h2 = h2sp.tile([P, MT], BF16, tag="h2")
                last_silu = nc.scalar.activation(
                    h2[:, :mt], hp[:, :mt], AF.Silu, bias=bt[2], scale=1.0
                )
                if pend is not None:
                    emit_z(*pend)
                pend = (h2, n0, mt)
                n0 += mt
            emit_z(*pend)

            # ---- Phase B: log_softmax, exp/ln interleaved (joint table set)
            eall = bigp.tile([P, NG * C], F32, tag="bigA", name="eall")

            exps = []
            g0 = 0
            for gn, se, oq in BCHUNKS:
                e = nc.scalar.activation(
                    eall[:, g0 * C : (g0 + gn) * C],
                    zall[:, g0 * C : (g0 + gn) * C],
                    AF.Exp,
                )
                exps.append((e, g0, gn, se, oq))
                g0 += gn
            add_dep_helper(exps[0][0].ins, last_silu.ins, sync=True,
                           reason="exp after all silus (ACT table switch)")

            sub_eng = [nc.gpsimd, nc.vector]
            out_q = [nc.sync, nc.scalar]
            for e, g0, gn, se, oq in exps:
                nc.vector.reduce_sum(
                    sall[:, g0 : g0 + gn],
                    eall[:, g0 * C : (g0 + gn) * C].rearrange(
                        "p (g c) -> p g c", g=gn
                    ),
                    axis=mybir.AxisListType.X,
                )
                nc.scalar.activation(
                    lsall[:, g0 : g0 + gn], sall[:, g0 : g0 + gn], AF.Ln
                )
                o = obp.tile([P, 28 * C], BF16, tag="o")
                sub_eng[se].tensor_tensor(
                    o[:, : gn * C].rearrange("p (g c) -> p g c", g=gn),
                    zall[:, g0 * C : (g0 + gn) * C].rearrange(
                        "p (g c) -> p g c", g=gn
                    ),
                    lsall[:, g0 : g0 + gn].broadcast_to([P, gn, C]),
                    op=mybir.AluOpType.subtract,
                )
                out_q[oq].dma_start(
                    out[:, g0 * C : (g0 + gn) * C], o[:, : gn * C]
                )
    nc.compile()
    _CACHE["nc"] = nc
    return nc


def _in_maps(x, W0, b0, W1, b1, W2, b2, W3, b3):
    import ml_dtypes

    x = np.asarray(x, dtype=np.float32)
    xpad = np.zeros((N_CORES * NS, P), dtype=ml_dtypes.bfloat16)
    xpad[:N_FULL] = x
    parts = [
        np.asarray(W0, np.float32).astype(ml_dtypes.bfloat16).view(np.uint8),
        np.asarray(W1, np.float32).astype(ml_dtypes.bfloat16).view(np.uint8),
        np.asarray(W2, np.float32).astype(ml_dtypes.bfloat16).view(np.uint8),
        np.asarray(W3, np.float32).astype(ml_dtypes.bfloat16).view(np.uint8),
        np.asarray(b0, np.float32).reshape(P, 1).view(np.uint8),
        np.asarray(b1, np.float32).reshape(P, 1).view(np.uint8),
        np.asarray(b2, np.float32).reshape(P, 1).view(np.uint8),
        np.ascontiguousarray(
            np.broadcast_to(np.tile(np.asarray(b3, np.float32), 12), (P, 12 * C))
        ).view(np.uint8),
    ]
    common = {"consts": np.ascontiguousarray(np.concatenate(parts, axis=1))}
    maps = []
    for c in range(N_CORES):
        shard = xpad[c * NS : (c + 1) * NS]
        maps.append({**common, "xT": np.ascontiguousarray(shard.T)})
    return maps


def _unscramble(res):
    # device out: bf16 [128, 196*40] with node = g*128 + p -> f32 [25088, 40]
    outs = []
    for c in range(N_CORES):
        o = res.results[c]["out"].reshape(P, NG, C).astype(np.float32)
        outs.append(np.ascontiguousarray(o.transpose(1, 0, 2)).reshape(NS, C))
    return np.concatenate(outs, axis=0)[:N_FULL]


def kernel(**inputs):
    nc = _build()
    maps = _in_maps(
        inputs["x"],
        inputs["W0"], inputs["b0"],
        inputs["W1"], inputs["b1"],
        inputs["W2"], inputs["b2"],
        inputs["W3"], inputs["b3"],
    )
    res = run_bass_kernel_spmd(nc, maps, list(range(N_CORES)))
    return _unscramble(res)
